# revision 58
# baseline (speedup 1.0000x reference)
"""NormLinearAttention kernel for 8 Trainium2 NeuronCores (bf16 build).

Strategy:
  Kernel A: head-parallel (core c owns head c, both batches). bf16 matmuls
    (1 cyc/row at any free size on TRN2). Projections: q in [hd, tok]
    layout; k|v paired in one token-layout matmul so v needs no transpose.
    Linear attention with mask = exp(s*tril):
        out_i = (e^s-1)*causal_i + full_i
    chunked at 128 (q blocks) with state snapshots every 256 tokens:
        Q_m = (e^s-1)*P_m + S_full  (precombined, bf16)
        ot  = v_blk @ (mask.et) [+ v_prev @ es1.et_off] + Q_m^T q
  Host reshards head-parallel y -> token-parallel for Kernel B.
  Kernel B: token-parallel (512 tokens/core): u proj, SRMSNorm via Rsqrt
    activation, z = (u+bu)*y, out proj, scale by rsqrt.

Both batches' phases are interleaved so DVE/Pool bookkeeping chains hide
under the other batch's PE work. DMAs are few and large (one per x tile).
"""

import sys

sys.path.insert(0, "/opt/trn_rl_repo")

import numpy as np
import ml_dtypes

import bass_rust
import concourse.bass as bass
import concourse.mybir as mybir
import concourse.tile as tile
from concourse.bass_utils import run_bass_kernel_spmd

F32 = mybir.dt.float32
BF16 = mybir.dt.bfloat16
AF = mybir.ActivationFunctionType
ALU = mybir.AluOpType

B, N, D = 2, 2048, 1024
H = 8
HD = 128
NC = 8
TT = 512           # projection token tile
NB = N // 128      # 16 token blocks per batch
NS = N // 256      # 8 state chunks per batch
EPS = 1e-6
BF_NP = ml_dtypes.bfloat16


def _split_multi_waits(nc, max_waits=1):
    """walrus accepts one sync wait per instruction; Tile emits several.
    Insert same-engine NoOps each carrying one of the extra waits."""
    ctr = 0
    for func in nc.m.functions:
        for blk in func.blocks:
            out = []
            changed = False
            for inst in blk.instructions:
                si = inst.sync_info
                if si is not None and len(si.on_wait) > max_waits:
                    waits = list(si.on_wait)
                    for w in waits[:-max_waits]:
                        nop = bass_rust.InstNoOp(name=f"WSPLIT-{ctr}", ins=[], outs=[])
                        ctr += 1
                        nop.engine = inst.engine
                        nop.sync_info = mybir.SyncInfo(on_wait=[w], on_update=[])
                        out.append(nop)
                    inst.sync_info = mybir.SyncInfo(
                        on_wait=waits[-max_waits:], on_update=list(si.on_update)
                    )
                    changed = True
                out.append(inst)
            if changed:
                blk.instructions = out
    return ctr


def build_kernel_a(dbg=False):
    nc = bass.Bass("TRN2", target_bir_lowering=False, debug=False, num_devices=NC)
    xh_in = nc.dram_tensor("xh", [B, 4, 128, 8 * TT], BF16, kind="ExternalInput").ap()
    wq_in = nc.dram_tensor("wq", [128, 8 * 128], BF16, kind="ExternalInput").ap()
    wkv_in = nc.dram_tensor("wkv", [128, 8 * 256], BF16, kind="ExternalInput").ap()
    cosT_in = nc.dram_tensor("cosT", [128, N], BF16, kind="ExternalInput").ap()
    sinT_in = nc.dram_tensor("sinT", [128, N], BF16, kind="ExternalInput").ap()
    cosTT_in = nc.dram_tensor("cosTT", [128, N], BF16, kind="ExternalInput").ap()
    sinTT_in = nc.dram_tensor("sinTT", [128, N], BF16, kind="ExternalInput").ap()
    mask3_in = nc.dram_tensor("mask3", [128, 384], F32, kind="ExternalInput").ap()
    bkv_in = nc.dram_tensor("bkv", [128, 512], BF16, kind="ExternalInput").ap()
    bq_in = nc.dram_tensor("bq", [128, 1], F32, kind="ExternalInput").ap()
    es1_in = nc.dram_tensor("es1", [128, 1], F32, kind="ExternalInput").ap()
    ident_in = nc.dram_tensor("ident", [128, 128], BF16, kind="ExternalInput").ap()
    y_out = nc.dram_tensor("y", [B, 128, N], BF16, kind="ExternalOutput").ap()
    dbg_outs = {}
    if dbg:
        for nm in ("qec", "qes", "kec", "kes", "knc", "kns", "ksn", "ksT"):
            dbg_outs[nm] = nc.dram_tensor(
                "d_" + nm, [128, N], BF16, kind="ExternalOutput").ap()
        dbg_outs["kvnat"] = nc.dram_tensor(
            "d_kvnat", [128, NB * 256], BF16, kind="ExternalOutput").ap()
        dbg_outs["QB"] = nc.dram_tensor(
            "d_QB", [128, NS * 256], BF16, kind="ExternalOutput").ap()

    with tile.TileContext(nc) as tc:
        with tc.tile_pool(name="consts", bufs=1) as cp, \
             tc.tile_pool(name="big", bufs=1) as bp, \
             tc.tile_pool(name="xt", bufs=8) as xp, \
             tc.tile_pool(name="qsil", bufs=3) as qsp, \
             tc.tile_pool(name="em", bufs=6) as emp:
            # wq first (smallest), then the first x tile in quarters so the
            # first projection matmuls start as early as possible
            xt00 = xp.tile([128, 8 * TT], BF16, tag="xt")
            wqt = cp.tile([128, 8 * 128], BF16)
            wkvt = cp.tile([128, 8 * 256], BF16)
            cosT = cp.tile([128, N], BF16)
            sinT = cp.tile([128, N], BF16)
            cosTT = cp.tile([128, N], BF16)
            sinTT = cp.tile([128, N], BF16)
            mask3 = cp.tile([128, 384], F32)
            bkv = cp.tile([128, 512], BF16)
            bq = cp.tile([128, 1], F32)
            es1 = cp.tile([128, 1], F32)
            ident = cp.tile([128, 128], BF16)
            bz = cp.tile([128, 1], F32)
            nc.sync.dma_start(wqt[:], wq_in)
            nc.sync.dma_start(xt00[:, 0:2 * TT], xh_in[0][0][:, 0:2 * TT])
            nc.sync.dma_start(xt00[:, 2 * TT:4 * TT], xh_in[0][0][:, 2 * TT:4 * TT])
            nc.sync.dma_start(wkvt[:], wkv_in)
            nc.sync.dma_start(xt00[:, 4 * TT:8 * TT], xh_in[0][0][:, 4 * TT:8 * TT])
            nc.sync.dma_start(bq[:], bq_in)
            nc.sync.dma_start(bkv[:], bkv_in)
            nc.sync.dma_start(es1[:], es1_in)
            nc.sync.dma_start(mask3[:], mask3_in)
            nc.sync.dma_start(ident[:], ident_in)
            nc.vector.memset(bz[:], 0.0)

            st = []  # per-batch persistent tensors
            for b in range(B):
                d = {}
                for nm, shape, dt in (
                    ("kvnat", [128, NB * 256], BF16),
                    ("ksn", [128, N], BF16),
                    ("ksT", [128, N], BF16),
                    ("qec", [128, N], BF16),
                    ("qes", [128, N], BF16),
                    ("kec", [128, N], BF16),
                    ("kes", [128, N], BF16),
                    ("knc", [128, N], BF16),
                    ("kns", [128, N], BF16),
                    ("QB", [128, NS * 256], BF16),
                    ("Pb", [128, 256], F32),
                    ("yst", [128, N], BF16),
                ):
                    d[nm] = bp.tile(shape, dt, tag=f"{nm}{b}", name=f"{nm}{b}")
                st.append(d)

            def vn_blk(b, j):
                # v for token block j lives in the kv-paired tile
                return st[b]["kvnat"][:, j * 256 + 128:(j + 1) * 256]

            def emit_tables(t):
                # stream each table's columns for one tile, so x tiles and
                # tables interleave on the DMA queue
                tsl = slice(t * TT, (t + 1) * TT)
                nc.sync.dma_start(cosT[:, tsl], cosT_in[:, tsl])
                nc.sync.dma_start(sinT[:, tsl], sinT_in[:, tsl])
                nc.sync.dma_start(cosTT[:, tsl], cosTT_in[:, tsl])
                nc.sync.dma_start(sinTT[:, tsl], sinTT_in[:, tsl])

            def phase1_tile(b, t, pq, pkv, ptp, xt=None):
                s = st[b]
                tsl = slice(t * TT, (t + 1) * TT)
                if xt is None:
                    xt = xp.tile([128, 8 * TT], BF16, tag="xt")
                    nc.sync.dma_start(xt[:], xh_in[b][t])
                # q projection: [hd, tok]
                psq = pq.tile([128, TT], F32, tag="psq")
                for e in range(8):
                    nc.tensor.matmul(
                        psq[:], wqt[:, e * 128:(e + 1) * 128],
                        xt[:, e * TT:(e + 1) * TT],
                        start=(e == 0), stop=(e == 7))
                qsil = qsp.tile([128, TT], BF16, tag="qsil")
                nc.scalar.activation(qsil[:], psq[:], AF.Silu, bias=bq[:])
                # k|v paired projection: [tok, k|v]; two token blocks share a
                # psum bank as sequential accumulation groups
                ptr4 = ptp.tile([128, 512], BF16, tag="ptr4")
                for jp in range(2):
                    pskv = pkv.tile([128, 512], F32, tag="pskv")
                    for half in range(2):
                        jb = jp * 2 + half
                        for e in range(8):
                            nc.tensor.matmul(
                                pskv[:, half * 256:(half + 1) * 256],
                                xt[:, e * TT + jb * 128: e * TT + (jb + 1) * 128],
                                wkvt[:, e * 256:(e + 1) * 256],
                                start=(e == 0), stop=(e == 7))
                    jj0 = t * 4 + jp * 2
                    nc.vector.tensor_add(
                        s["kvnat"][:, jj0 * 256:(jj0 + 2) * 256], pskv[:], bkv[:])
                    kpair = s["kvnat"].rearrange("p (j kv c) -> p j kv c",
                                                 kv=2, c=128)
                    nc.scalar.activation(
                        s["ksn"][:, jj0 * 128:(jj0 + 2) * 128],
                        kpair[:, jj0:jj0 + 2, 0],
                        AF.Silu, bias=bz[:])
                    for half in range(2):
                        jb = jp * 2 + half
                        jj = t * 4 + jb
                        nc.tensor.transpose(ptr4[:, jb * 128:(jb + 1) * 128],
                                            s["ksn"][:, jj * 128:(jj + 1) * 128],
                                            ident[:])
                nc.scalar.copy(s["ksT"][:, tsl], ptr4[:])
                # lrpe, state-path (knc/kns) first: the interleaved 2a chunk
                # matmuls wait on them; q/kec/kes are only read in 2b
                nc.vector.tensor_mul(s["knc"][:, tsl], s["ksn"][:, tsl], cosTT[:, tsl])
                nc.vector.tensor_mul(s["kns"][:, tsl], s["ksn"][:, tsl], sinTT[:, tsl])
                nc.vector.tensor_mul(s["kec"][:, tsl], s["ksT"][:, tsl], cosT[:, tsl])
                nc.vector.tensor_mul(s["kes"][:, tsl], s["ksT"][:, tsl], sinT[:, tsl])
                nc.vector.tensor_mul(s["qec"][:, tsl], qsil[:], cosT[:, tsl])
                nc.vector.tensor_mul(s["qes"][:, tsl], qsil[:], sinT[:, tsl])

            def phase2a_chunk(b, ms, pst):
                s = st[b]
                if ms == 0:
                    nc.gpsimd.memset(s["Pb"][:], 0.0)
                else:
                    # snapshot (e^s-1)*P_m before adding chunk ms
                    nc.gpsimd.tensor_scalar_mul(
                        s["QB"][:, ms * 256:(ms + 1) * 256], s["Pb"][:], es1[:])
                scb = pst.tile([128, 256], F32, tag="scb")
                # two sequential accumulation groups sharing one bank
                for g in range(2):
                    kn = s["knc"] if g == 0 else s["kns"]
                    for s2 in range(2):
                        j = 2 * ms + s2
                        bsl = slice(j * 128, (j + 1) * 128)
                        nc.tensor.matmul(scb[:, g * 128:(g + 1) * 128],
                                         kn[:, bsl], vn_blk(b, j),
                                         start=(s2 == 0), stop=(s2 == 1))
                # gpsimd cannot touch PSUM on hw; this add reads scb (PSUM)
                nc.vector.tensor_add(s["Pb"][:], s["Pb"][:], scb[:])

            def phase2a_post(b):
                s = st[b]
                # Q_0 = S_full; Q_m += S_full
                nc.gpsimd.tensor_copy(s["QB"][:, 0:256], s["Pb"][:])
                for ms in range(1, NS):
                    msl = slice(ms * 256, (ms + 1) * 256)
                    nc.gpsimd.tensor_add(s["QB"][:, msl], s["QB"][:, msl],
                                         s["QB"][:, 0:256])

            def phase2b(b, pet, pot):
                s = st[b]

                def emit_et(ms):
                    etb = pet.tile([128, 512], F32, tag="pskv", name=f"etb{ms}")
                    for s2 in range(2):
                        qb = 2 * ms + s2
                        qsl = slice(qb * 128, (qb + 1) * 128)
                        et = etb[:, s2 * 128:(s2 + 1) * 128]
                        nc.tensor.matmul(et, s["kec"][:, qsl], s["qec"][:, qsl],
                                         start=True, stop=False)
                        nc.tensor.matmul(et, s["kes"][:, qsl], s["qes"][:, qsl],
                                         start=False, stop=True)
                    bsl = slice(2 * ms * 128, (2 * ms + 1) * 128)
                    q1sl = slice((2 * ms + 1) * 128, (2 * ms + 2) * 128)
                    et2 = etb[:, 256:384]
                    nc.tensor.matmul(et2, s["kec"][:, bsl], s["qec"][:, q1sl],
                                     start=True, stop=False)
                    nc.tensor.matmul(et2, s["kes"][:, bsl], s["qes"][:, q1sl],
                                     start=False, stop=True)
                    # one masked multiply for all three energy blocks (DVE is
                    # idle during 2b; Pool owns the state bookkeeping)
                    em3 = emp.tile([128, 384], BF16, tag="em3", name=f"em3_{ms}")
                    nc.vector.tensor_mul(em3[:], etb[:, 0:384], mask3[:])
                    return em3

                def emit_ot(ms, em3):
                    qmsl = slice(ms * 256, ms * 256 + 128)
                    smsl = slice(ms * 256 + 128, (ms + 1) * 256)
                    otb = pot.tile([128, 256], F32, tag="scb", name=f"otb{ms}")
                    for s2 in range(2):
                        qb = 2 * ms + s2
                        qsl = slice(qb * 128, (qb + 1) * 128)
                        ot = otb[:, s2 * 128:(s2 + 1) * 128]
                        nc.tensor.matmul(ot, s["QB"][:, qmsl], s["qec"][:, qsl],
                                         start=True, stop=False)
                        nc.tensor.matmul(ot, s["QB"][:, smsl], s["qes"][:, qsl],
                                         start=False, stop=False)
                        if s2 == 1:
                            nc.tensor.matmul(ot, vn_blk(b, 2 * ms),
                                             em3[:, 256:384],
                                             start=False, stop=False)
                        nc.tensor.matmul(ot, vn_blk(b, qb),
                                         em3[:, s2 * 128:(s2 + 1) * 128],
                                         start=False, stop=True)
                    nc.scalar.copy(s["yst"][:, 2 * ms * 128:(2 * ms + 2) * 128],
                                   otb[:])
                    if ms % 2 == 1:
                        g = ms // 2
                        gsl = slice(g * 512, (g + 1) * 512)
                        nc.sync.dma_start(y_out[b][:, gsl], s["yst"][:, gsl])

                # software pipeline: energies run two chunks ahead of outputs
                ems = {}
                for ms in range(NS):
                    ems[ms] = emit_et(ms)
                    if ms >= 2:
                        emit_ot(ms - 2, ems.pop(ms - 2))
                emit_ot(NS - 2, ems.pop(NS - 2))
                emit_ot(NS - 1, ems.pop(NS - 1))

            with tc.tile_pool(name="pq", bufs=2, space="PSUM") as pq, \
                 tc.tile_pool(name="pkv", bufs=3, space="PSUM") as pkv, \
                 tc.tile_pool(name="ptr", bufs=1, space="PSUM") as ptp, \
                 tc.tile_pool(name="pst", bufs=2, space="PSUM") as pst:
                # prefetch b0's x tiles interleaved with table columns
                xts0 = [xt00]
                for t in range(1, 4):
                    xtn = xp.tile([128, 8 * TT], BF16, tag="xt", name=f"xt0{t}")
                    nc.sync.dma_start(xtn[:], xh_in[0][t])
                    emit_tables(t - 1)
                    xts0.append(xtn)
                emit_tables(3)
                # prefetch b1's x tiles too (DMA track is idle during b0)
                xts1 = []
                for t in range(4):
                    xtn = xp.tile([128, 8 * TT], BF16, tag="xt", name=f"xt1{t}")
                    nc.sync.dma_start(xtn[:], xh_in[1][t])
                    xts1.append(xtn)
                # state chunks interleave with projection tiles: chunk 2t
                # and 2t+1 only need tile t's outputs, so the serial P chain
                # advances under the next tile's matmuls
                for t in range(4):
                    phase1_tile(0, t, pq, pkv, ptp, xt=xts0[t])
                    phase2a_chunk(0, 2 * t, pst)
                    phase2a_chunk(0, 2 * t + 1, pst)
                phase2a_post(0)
                for t in range(4):
                    phase1_tile(1, t, pq, pkv, ptp, xt=xts1[t])
                    phase2a_chunk(1, 2 * t, pst)
                    phase2a_chunk(1, 2 * t + 1, pst)
                phase2a_post(1)
                # 2b reuses phase-1 psum buffers (same shapes, same tags) so
                # no pool-release barrier sits between the phases
                phase2b(0, pkv, pst)
                phase2b(1, pkv, pst)
            if dbg:
                for nm, dst in dbg_outs.items():
                    nc.sync.dma_start(dst, st[0][nm][:])

    return nc


def build_kernel_b():
    nc = bass.Bass("TRN2", target_bir_lowering=False, debug=False, num_devices=NC)
    NT = B * N // NC  # 512 tokens per core
    yt_in = nc.dram_tensor("ytx", [128, H * NT], BF16, kind="ExternalInput").ap()
    xs_in = nc.dram_tensor("xs", [128, 8 * NT], BF16, kind="ExternalInput").ap()
    wu_in = nc.dram_tensor("wu", [128, 64 * 128], BF16, kind="ExternalInput").ap()
    wo_in = nc.dram_tensor("wo", [128, 64 * 128], BF16, kind="ExternalInput").ap()
    bu_in = nc.dram_tensor("bu", [128, 8], F32, kind="ExternalInput").ap()
    ones_in = nc.dram_tensor("ones", [128, 128], BF16, kind="ExternalInput").ap()
    out_t = nc.dram_tensor("outT", [128, 8 * NT], BF16, kind="ExternalOutput").ap()

    with tile.TileContext(nc) as tc:
        with tc.tile_pool(name="ins", bufs=1) as ip, \
             tc.tile_pool(name="work", bufs=1) as wk, \
             tc.tile_pool(name="y2p", bufs=3) as y2p, \
             tc.tile_pool(name="pu", bufs=4, space="PSUM") as pup, \
             tc.tile_pool(name="po", bufs=3, space="PSUM") as pop, \
             tc.tile_pool(name="pss", bufs=1, space="PSUM") as pss:
            yt = ip.tile([128, H * NT], BF16)
            xs = ip.tile([128, 8 * NT], BF16)
            wut = ip.tile([128, 64 * 128], BF16)
            wot = ip.tile([128, 64 * 128], BF16)
            bu = ip.tile([128, 8], F32)
            ones = ip.tile([128, 128], BF16)
            # DMA order tuned so u-proj starts asap and each consumer's
            # input lands just ahead of its first use
            nc.sync.dma_start(xs[:, 0:4 * NT], xs_in[:, 0:4 * NT])
            nc.sync.dma_start(ones[:], ones_in)
            nc.sync.dma_start(bu[:], bu_in)
            nc.sync.dma_start(wut[:, 0:2048], wu_in[:, 0:2048])
            nc.sync.dma_start(wut[:, 2048:4096], wu_in[:, 2048:4096])
            nc.sync.dma_start(xs[:, 4 * NT:8 * NT], xs_in[:, 4 * NT:8 * NT])
            nc.sync.dma_start(yt[:, 0:4 * NT], yt_in[:, 0:4 * NT])
            nc.sync.dma_start(wut[:, 4096:6144], wu_in[:, 4096:6144])
            nc.sync.dma_start(wut[:, 6144:8192], wu_in[:, 6144:8192])
            nc.sync.dma_start(yt[:, 4 * NT:8 * NT], yt_in[:, 4 * NT:8 * NT])
            for q in range(4):
                nc.sync.dma_start(wot[:, q * 2048:(q + 1) * 2048],
                                  wo_in[:, q * 2048:(q + 1) * 2048])

            # squares queue on Act early (they only need yt halves)
            y2s = []
            for hc in range(H):
                y2 = y2p.tile([128, NT], BF16, tag=f"y2_{hc}", name=f"y2_{hc}")
                nc.scalar.activation(y2[:], yt[:, hc * NT:(hc + 1) * NT], AF.Square)
                y2s.append(y2)

            def wsl(w, a, bb):
                o = (a * 8 + bb) * 128
                return w[:, o:o + 128]

            z = wk.tile([128, 8 * NT], BF16, tag="z")
            ostage = wk.tile([128, 8 * NT], BF16, tag="ostage")
            HT = NT // 2

            def u_half(hf):
                def hslc(base):
                    return slice(base * NT + hf * HT, base * NT + (hf + 1) * HT)
                for uc in range(8):
                    ps = pup.tile([128, HT], F32, tag="pu")
                    for e in range(8):
                        nc.tensor.matmul(ps[:], wsl(wut, uc, e), xs[:, hslc(e)],
                                         start=(e == 0), stop=(e == 7))
                    nc.vector.scalar_tensor_tensor(
                        z[:, hslc(uc)], ps[:], bu[:, uc:uc + 1],
                        yt[:, hslc(uc)], ALU.add, ALU.mult)

            def o_half(hf):
                def hslc(base):
                    return slice(base * NT + hf * HT, base * NT + (hf + 1) * HT)
                for oc in range(8):
                    ps = pop.tile([128, HT], F32, tag="po")
                    for e in range(8):
                        nc.tensor.matmul(ps[:], wsl(wot, oc, e), z[:, hslc(e)],
                                         start=(e == 0), stop=(e == 7))
                    nc.vector.tensor_mul(ostage[:, hslc(oc)], ps[:],
                                         rs2[:, hf * HT:(hf + 1) * HT])
                    if oc % 2 == 1:
                        # ship each finished pair of output blocks
                        o_r = out_t.rearrange("p (a t) -> p a t", a=8)
                        s_r = ostage.rearrange("p (a t) -> p a t", a=8)
                        hsl2 = slice(hf * HT, (hf + 1) * HT)
                        nc.sync.dma_start(o_r[:, oc - 1:oc + 1, hsl2],
                                          s_r[:, oc - 1:oc + 1, hsl2])

            # first token-half u-proj with a split contraction: uc0-3 start
            # on the first xs half + wu quarter while the rest streams in
            def hslc0(base):
                return slice(base * NT, base * NT + HT)
            open_ps = []
            for uc in range(4):
                ps = pup.tile([128, HT], F32, tag="pu", name=f"pu_s{uc}")
                for e in range(4):
                    nc.tensor.matmul(ps[:], wsl(wut, uc, e), xs[:, hslc0(e)],
                                     start=(e == 0), stop=False)
                open_ps.append(ps)
            for uc in range(4):
                ps = open_ps[uc]
                for e in range(4, 8):
                    nc.tensor.matmul(ps[:], wsl(wut, uc, e), xs[:, hslc0(e)],
                                     start=False, stop=(e == 7))
                nc.vector.scalar_tensor_tensor(
                    z[:, hslc0(uc)], ps[:], bu[:, uc:uc + 1],
                    yt[:, hslc0(uc)], ALU.add, ALU.mult)
            for uc in range(4, 8):
                ps = pup.tile([128, HT], F32, tag="pu")
                for e in range(8):
                    nc.tensor.matmul(ps[:], wsl(wut, uc, e), xs[:, hslc0(e)],
                                     start=(e == 0), stop=(e == 7))
                nc.vector.scalar_tensor_tensor(
                    z[:, hslc0(uc)], ps[:], bu[:, uc:uc + 1],
                    yt[:, hslc0(uc)], ALU.add, ALU.mult)
            # ssq + norm chain between the u and o halves: PE fills with
            # these cheap matmuls while wo still streams in
            ssq = pss.tile([128, NT], F32, tag="ssq")
            for hc in range(H):
                nc.tensor.matmul(ssq[:], ones[:], y2s[hc][:],
                                 start=(hc == 0), stop=(hc == H - 1))
            # rsqrt(ms+eps) = exp(-0.5*ln(ms+eps)); one Newton step for accuracy
            xb = wk.tile([128, NT], F32, tag="xb")
            nc.vector.tensor_scalar(xb[:], ssq[:], 1.0 / D, EPS, ALU.mult, ALU.add)
            bz = ip.tile([128, 1], F32)
            nc.vector.memset(bz[:], 0.0)
            lnv = wk.tile([128, NT], F32, tag="lnv")
            nc.scalar.activation(lnv[:], xb[:], AF.Ln, bias=bz[:])
            rsb = wk.tile([128, NT], F32, tag="rsb")
            nc.scalar.activation(rsb[:], lnv[:], AF.Exp, bias=bz[:], scale=-0.5)
            t1 = wk.tile([128, NT], F32, tag="t1")
            nc.vector.tensor_mul(t1[:], xb[:], rsb[:])
            nc.vector.tensor_mul(t1[:], t1[:], rsb[:])
            nc.vector.tensor_scalar(t1[:], t1[:], -0.5, 1.5, ALU.mult, ALU.add)
            rs2 = wk.tile([128, NT], F32, tag="rs2")
            nc.vector.tensor_mul(rs2[:], rsb[:], t1[:])
            u_half(1)
            o_half(0)
            o_half(1)

    return nc


_CACHE = {}


def _bf(a):
    return np.ascontiguousarray(a.astype(BF_NP))


def kernel(x, slope_rate, qkvu_w, qkvu_b, out_w, out_b, theta):
    x = np.asarray(x, np.float32)
    slope_rate = np.asarray(slope_rate, np.float32)
    qkvu_w = np.asarray(qkvu_w, np.float32)
    qkvu_b = np.asarray(qkvu_b, np.float32)
    out_w = np.asarray(out_w, np.float32)
    out_b = np.asarray(out_b, np.float32)
    theta = np.asarray(theta, np.float32)

    # x in [b, t, p, e*512+j] layout: xh[b,t,p,e*512+j] = x[b, t*512+j, e*128+p]
    xh = _bf(x.reshape(B, 4, TT, 8, 128).transpose(0, 1, 4, 3, 2)
             .reshape(B, 4, 128, 8 * TT))
    idx = np.arange(N, dtype=np.float32)
    ident = np.eye(128, dtype=np.float32)

    in_maps_a = []
    for c in range(NC):
        th = theta[c, 0].astype(np.float32)[:, None] * idx[None, :]  # [128, N]
        es = np.exp(slope_rate[c, 0, 0]).astype(np.float32)
        es1 = np.float32(es - 1.0)
        sl = slice(c * HD, (c + 1) * HD)
        wq = qkvu_w[0 * D:1 * D][sl]   # [128, D]
        wk = qkvu_w[1 * D:2 * D][sl]
        wv = qkvu_w[2 * D:3 * D][sl]
        # wq image: [p, e*128+o] = wq[o, e*128+p]
        wq_img = wq.T.reshape(8, 128, 128).transpose(1, 0, 2).reshape(128, 8 * 128)
        # wkv image: [p, e*256 + (k|v 128+o)]
        wkv_img = np.concatenate(
            [wk.T.reshape(8, 128, 1, 128), wv.T.reshape(8, 128, 1, 128)], axis=2
        ).transpose(1, 0, 2, 3).reshape(128, 8 * 256)
        cosv = np.cos(th)  # [hd, pos]
        sinv = np.sin(th)
        # token-layout tables: [p, jj*128 + d] = f(theta_d * (jj*128+p))
        cosvT = cosv.T.reshape(NB, 128, 128).transpose(1, 0, 2).reshape(128, N)
        sinvT = sinv.T.reshape(NB, 128, 128).transpose(1, 0, 2).reshape(128, N)
        m0 = (np.arange(128)[:, None] <= np.arange(128)[None, :]).astype(np.float32)
        m3 = np.concatenate(
            [m0 * es1, m0 * es1, np.full((128, 128), es1, np.float32)], axis=1)
        bkv_img = np.broadcast_to(
            np.concatenate([qkvu_b[1 * D:2 * D][sl], qkvu_b[2 * D:3 * D][sl]] * 2),
            (128, 512))
        in_maps_a.append({
            "xh": xh,
            "wq": _bf(wq_img),
            "wkv": _bf(wkv_img),
            "cosT": _bf(cosv),
            "sinT": _bf(sinv),
            "cosTT": _bf(cosvT),
            "sinTT": _bf(sinvT),
            "mask3": np.ascontiguousarray(m3),
            "bkv": _bf(bkv_img),
            "bq": np.ascontiguousarray(qkvu_b[0 * D:1 * D][sl].reshape(128, 1)),
            "es1": np.full((128, 1), es1, np.float32),
            "ident": _bf(ident),
        })

    if "a" not in _CACHE:
        _CACHE["a"] = build_kernel_a()
    nca = _CACHE["a"]
    if not getattr(nca, "_wsplit_done", False):
        _split_multi_waits(nca)
        nca._wsplit_done = True
    res_a = run_bass_kernel_spmd(nca, in_maps_a, list(range(NC))).results

    # reshard: core d of kernel B gets flat tokens [d*512, (d+1)*512)
    NT = B * N // NC
    xT = x.reshape(B * N, D).T  # [D, B*N]
    wu = qkvu_w[3 * D:4 * D]
    wu_img = wu.reshape(8, 128, 8, 128).transpose(3, 0, 2, 1).reshape(
        128, 64 * 128)  # [p, (uc*8+e)*128+o] = wu[uc*128+o, e*128+p]
    wo_img = out_w.reshape(8, 128, 8, 128).transpose(3, 0, 2, 1).reshape(
        128, 64 * 128)
    bu_img = qkvu_b[3 * D:4 * D].reshape(8, 128).T  # [o, uc]
    onesm = np.ones((128, 128), np.float32)

    in_maps_b = []
    for d in range(NC):
        bb, off = d // 4, (d % 4) * NT
        ytx = np.concatenate(
            [np.asarray(res_a[h]["y"][bb][:, off:off + NT]) for h in range(H)],
            axis=1)  # [128, H*NT] bf16
        xsl = xT[:, d * NT:(d + 1) * NT].reshape(8, 128, NT).transpose(
            1, 0, 2).reshape(128, 8 * NT)
        in_maps_b.append({
            "ytx": np.ascontiguousarray(ytx),
            "xs": _bf(xsl),
            "wu": _bf(wu_img),
            "wo": _bf(wo_img),
            "bu": np.ascontiguousarray(bu_img),
            "ones": _bf(onesm),
        })

    if "b" not in _CACHE:
        _CACHE["b"] = build_kernel_b()
    ncb = _CACHE["b"]
    if not getattr(ncb, "_wsplit_done", False):
        _split_multi_waits(ncb)
        ncb._wsplit_done = True
    res_b = run_bass_kernel_spmd(ncb, in_maps_b, list(range(NC))).results

    out = np.empty((B * N, D), np.float32)
    for d in range(NC):
        o = np.asarray(res_b[d]["outT"]).astype(np.float32)  # [128, 8*NT]
        out[d * NT:(d + 1) * NT] = o.reshape(128, 8, NT).transpose(
            2, 1, 0).reshape(NT, D)
    out += out_b[None, :]
    return out.reshape(B, N, D)


# revision 74
# speedup vs baseline: 1.0284x; 1.0284x over previous
"""NormLinearAttention kernel for 8 Trainium2 NeuronCores (bf16 build).

Strategy:
  Kernel A: head-parallel (core c owns head c, both batches). bf16 matmuls
    (1 cyc/row at any free size on TRN2). Projections: q in [hd, tok]
    layout; k|v paired in one token-layout matmul so v needs no transpose.
    Linear attention with mask = exp(s*tril):
        out_i = (e^s-1)*causal_i + full_i
    chunked at 128 (q blocks) with state snapshots every 256 tokens:
        Q_m = (e^s-1)*P_m + S_full  (precombined, bf16)
        ot  = v_blk @ (mask.et) [+ v_prev @ es1.et_off] + Q_m^T q
  Host reshards head-parallel y -> token-parallel for Kernel B.
  Kernel B: token-parallel (512 tokens/core): u proj, SRMSNorm via Rsqrt
    activation, z = (u+bu)*y, out proj, scale by rsqrt.

Both batches' phases are interleaved so DVE/Pool bookkeeping chains hide
under the other batch's PE work. DMAs are few and large (one per x tile).
"""

import sys

sys.path.insert(0, "/opt/trn_rl_repo")

import numpy as np
import ml_dtypes

import bass_rust
import concourse.bass as bass
import concourse.mybir as mybir
import concourse.tile as tile
from concourse.bass_utils import run_bass_kernel_spmd

F32 = mybir.dt.float32
BF16 = mybir.dt.bfloat16
AF = mybir.ActivationFunctionType
ALU = mybir.AluOpType

B, N, D = 2, 2048, 1024
H = 8
HD = 128
NC = 8
TT = 512           # projection token tile
NB = N // 128      # 16 token blocks per batch
NS = N // 256      # 8 state chunks per batch
EPS = 1e-6
BF_NP = ml_dtypes.bfloat16


def _split_multi_waits(nc, max_waits=1):
    """walrus accepts one sync wait per instruction; Tile emits several.
    Insert same-engine NoOps each carrying one of the extra waits."""
    ctr = 0
    for func in nc.m.functions:
        for blk in func.blocks:
            out = []
            changed = False
            for inst in blk.instructions:
                si = inst.sync_info
                if si is not None and len(si.on_wait) > max_waits:
                    waits = list(si.on_wait)
                    for w in waits[:-max_waits]:
                        nop = bass_rust.InstNoOp(name=f"WSPLIT-{ctr}", ins=[], outs=[])
                        ctr += 1
                        nop.engine = inst.engine
                        nop.sync_info = mybir.SyncInfo(on_wait=[w], on_update=[])
                        out.append(nop)
                    inst.sync_info = mybir.SyncInfo(
                        on_wait=waits[-max_waits:], on_update=list(si.on_update)
                    )
                    changed = True
                out.append(inst)
            if changed:
                blk.instructions = out
    return ctr


def build_kernel_a(dbg=False):
    nc = bass.Bass("TRN2", target_bir_lowering=False, debug=False, num_devices=NC)
    xh_in = nc.dram_tensor("xh", [B, 4, 128, 8 * TT], BF16, kind="ExternalInput").ap()
    wq_in = nc.dram_tensor("wq", [128, 8 * 128], BF16, kind="ExternalInput").ap()
    wkv_in = nc.dram_tensor("wkv", [128, 8 * 256], BF16, kind="ExternalInput").ap()
    cosT_in = nc.dram_tensor("cosT", [128, N], BF16, kind="ExternalInput").ap()
    sinT_in = nc.dram_tensor("sinT", [128, N], BF16, kind="ExternalInput").ap()
    cosTT_in = nc.dram_tensor("cosTT", [128, N], BF16, kind="ExternalInput").ap()
    sinTT_in = nc.dram_tensor("sinTT", [128, N], BF16, kind="ExternalInput").ap()
    mask3_in = nc.dram_tensor("mask3", [128, 384], F32, kind="ExternalInput").ap()
    bkv_in = nc.dram_tensor("bkv", [128, 512], BF16, kind="ExternalInput").ap()
    bq_in = nc.dram_tensor("bq", [128, 1], F32, kind="ExternalInput").ap()
    es1_in = nc.dram_tensor("es1", [128, 1], F32, kind="ExternalInput").ap()
    ident_in = nc.dram_tensor("ident", [128, 128], BF16, kind="ExternalInput").ap()
    y_out = nc.dram_tensor("y", [B, 128, N], BF16, kind="ExternalOutput").ap()
    dbg_outs = {}
    if dbg:
        for nm in ("qec", "qes", "kec", "kes", "knc", "kns", "ksn", "ksT"):
            dbg_outs[nm] = nc.dram_tensor(
                "d_" + nm, [128, N], BF16, kind="ExternalOutput").ap()
        dbg_outs["kvnat"] = nc.dram_tensor(
            "d_kvnat", [128, NB * 256], BF16, kind="ExternalOutput").ap()
        dbg_outs["QB"] = nc.dram_tensor(
            "d_QB", [128, NS * 256], BF16, kind="ExternalOutput").ap()

    with tile.TileContext(nc) as tc:
        with tc.tile_pool(name="consts", bufs=1) as cp, \
             tc.tile_pool(name="big", bufs=1) as bp, \
             tc.tile_pool(name="xt", bufs=8) as xp, \
             tc.tile_pool(name="qsil", bufs=3) as qsp, \
             tc.tile_pool(name="em", bufs=6) as emp:
            # wq first (smallest), then the first x tile in quarters so the
            # first projection matmuls start as early as possible
            xt00 = xp.tile([128, 8 * TT], BF16, tag="xt")
            wqt = cp.tile([128, 8 * 128], BF16)
            wkvt = cp.tile([128, 8 * 256], BF16)
            cosT = cp.tile([128, N], BF16)
            sinT = cp.tile([128, N], BF16)
            cosTT = cp.tile([128, N], BF16)
            sinTT = cp.tile([128, N], BF16)
            mask3 = cp.tile([128, 384], F32)
            bkv = cp.tile([128, 512], BF16)
            bq = cp.tile([128, 1], F32)
            es1 = cp.tile([128, 1], F32)
            ident = cp.tile([128, 128], BF16)
            bz = cp.tile([128, 1], F32)
            nc.sync.dma_start(wqt[:], wq_in)
            nc.sync.dma_start(xt00[:, 0:2 * TT], xh_in[0][0][:, 0:2 * TT])
            nc.sync.dma_start(xt00[:, 2 * TT:4 * TT], xh_in[0][0][:, 2 * TT:4 * TT])
            nc.sync.dma_start(wkvt[:], wkv_in)
            nc.sync.dma_start(xt00[:, 4 * TT:8 * TT], xh_in[0][0][:, 4 * TT:8 * TT])
            nc.sync.dma_start(bq[:], bq_in)
            nc.sync.dma_start(bkv[:], bkv_in)
            nc.sync.dma_start(es1[:], es1_in)
            nc.sync.dma_start(mask3[:], mask3_in)
            nc.sync.dma_start(ident[:], ident_in)
            nc.vector.memset(bz[:], 0.0)
            # warm the PE p-state ramp while the first DMAs land: throwaway
            # matmuls on a zeroed tile so real matmuls start at full speed
            wz = qsp.tile([128, TT], BF16, tag="warm")
            nc.vector.memset(wz[:], 0.0)

            st = []  # per-batch persistent tensors
            for b in range(B):
                d = {}
                for nm, shape, dt in (
                    ("kvnat", [128, NB * 256], BF16),
                    ("ksn", [128, N], BF16),
                    ("ksT", [128, N], BF16),
                    ("qec", [128, N], BF16),
                    ("qes", [128, N], BF16),
                    ("kec", [128, N], BF16),
                    ("kes", [128, N], BF16),
                    ("knc", [128, N], BF16),
                    ("kns", [128, N], BF16),
                    ("QB", [128, NS * 256], BF16),
                    ("Pb", [128, 256], F32),
                    ("yst", [128, N], BF16),
                ):
                    d[nm] = bp.tile(shape, dt, tag=f"{nm}{b}", name=f"{nm}{b}")
                st.append(d)

            def vn_blk(b, j):
                # v for token block j lives in the kv-paired tile
                return st[b]["kvnat"][:, j * 256 + 128:(j + 1) * 256]

            def emit_tables(t):
                # stream each table's columns for one tile, so x tiles and
                # tables interleave on the DMA queue
                tsl = slice(t * TT, (t + 1) * TT)
                nc.sync.dma_start(cosT[:, tsl], cosT_in[:, tsl])
                nc.sync.dma_start(sinT[:, tsl], sinT_in[:, tsl])
                nc.sync.dma_start(cosTT[:, tsl], cosTT_in[:, tsl])
                nc.sync.dma_start(sinTT[:, tsl], sinTT_in[:, tsl])

            def phase1_tile(b, t, pq, pkv, ptp, xt=None):
                s = st[b]
                tsl = slice(t * TT, (t + 1) * TT)
                if xt is None:
                    xt = xp.tile([128, 8 * TT], BF16, tag="xt")
                    nc.sync.dma_start(xt[:], xh_in[b][t])
                # q projection: [hd, tok]
                psq = pq.tile([128, TT], F32, tag="psq")
                for e in range(8):
                    nc.tensor.matmul(
                        psq[:], wqt[:, e * 128:(e + 1) * 128],
                        xt[:, e * TT:(e + 1) * TT],
                        start=(e == 0), stop=(e == 7))
                qsil = qsp.tile([128, TT], BF16, tag="qsil")
                nc.scalar.activation(qsil[:], psq[:], AF.Silu, bias=bq[:])
                # k|v paired projection: [tok, k|v]; two token blocks share a
                # psum bank as sequential accumulation groups
                ptr4 = ptp.tile([128, 512], BF16, tag="ptr4")
                for jp in range(2):
                    pskv = pkv.tile([128, 512], F32, tag="pskv")
                    for half in range(2):
                        jb = jp * 2 + half
                        for e in range(8):
                            nc.tensor.matmul(
                                pskv[:, half * 256:(half + 1) * 256],
                                xt[:, e * TT + jb * 128: e * TT + (jb + 1) * 128],
                                wkvt[:, e * 256:(e + 1) * 256],
                                start=(e == 0), stop=(e == 7))
                    jj0 = t * 4 + jp * 2
                    nc.vector.tensor_add(
                        s["kvnat"][:, jj0 * 256:(jj0 + 2) * 256], pskv[:], bkv[:])
                    kpair = s["kvnat"].rearrange("p (j kv c) -> p j kv c",
                                                 kv=2, c=128)
                    nc.scalar.activation(
                        s["ksn"][:, jj0 * 128:(jj0 + 2) * 128],
                        kpair[:, jj0:jj0 + 2, 0],
                        AF.Silu, bias=bz[:])
                    for half in range(2):
                        jb = jp * 2 + half
                        jj = t * 4 + jb
                        nc.tensor.transpose(ptr4[:, jb * 128:(jb + 1) * 128],
                                            s["ksn"][:, jj * 128:(jj + 1) * 128],
                                            ident[:])
                nc.scalar.copy(s["ksT"][:, tsl], ptr4[:])
                # lrpe, state-path (knc/kns) first: the interleaved 2a chunk
                # matmuls wait on them; q/kec/kes are only read in 2b
                nc.vector.tensor_mul(s["knc"][:, tsl], s["ksn"][:, tsl], cosTT[:, tsl])
                nc.vector.tensor_mul(s["kns"][:, tsl], s["ksn"][:, tsl], sinTT[:, tsl])
                nc.vector.tensor_mul(s["kec"][:, tsl], s["ksT"][:, tsl], cosT[:, tsl])
                nc.vector.tensor_mul(s["kes"][:, tsl], s["ksT"][:, tsl], sinT[:, tsl])
                nc.vector.tensor_mul(s["qec"][:, tsl], qsil[:], cosT[:, tsl])
                nc.vector.tensor_mul(s["qes"][:, tsl], qsil[:], sinT[:, tsl])

            def phase2a_chunk(b, ms, pst):
                s = st[b]
                if ms == 0:
                    nc.gpsimd.memset(s["Pb"][:], 0.0)
                else:
                    # snapshot (e^s-1)*P_m before adding chunk ms
                    nc.gpsimd.tensor_scalar_mul(
                        s["QB"][:, ms * 256:(ms + 1) * 256], s["Pb"][:], es1[:])
                scb = pst.tile([128, 256], F32, tag="scb")
                # two sequential accumulation groups sharing one bank
                for g in range(2):
                    kn = s["knc"] if g == 0 else s["kns"]
                    for s2 in range(2):
                        j = 2 * ms + s2
                        bsl = slice(j * 128, (j + 1) * 128)
                        nc.tensor.matmul(scb[:, g * 128:(g + 1) * 128],
                                         kn[:, bsl], vn_blk(b, j),
                                         start=(s2 == 0), stop=(s2 == 1))
                # gpsimd cannot touch PSUM on hw; this add reads scb (PSUM)
                nc.vector.tensor_add(s["Pb"][:], s["Pb"][:], scb[:])

            def phase2a_post(b):
                s = st[b]
                # Q_0 = S_full; Q_m += S_full
                nc.gpsimd.tensor_copy(s["QB"][:, 0:256], s["Pb"][:])
                for ms in range(1, NS):
                    msl = slice(ms * 256, (ms + 1) * 256)
                    nc.gpsimd.tensor_add(s["QB"][:, msl], s["QB"][:, msl],
                                         s["QB"][:, 0:256])

            def phase2b(b, pet, pot):
                s = st[b]

                def emit_et(ms):
                    etb = pet.tile([128, 512], F32, tag="etb", name=f"etb{ms}")
                    for s2 in range(2):
                        qb = 2 * ms + s2
                        qsl = slice(qb * 128, (qb + 1) * 128)
                        et = etb[:, s2 * 128:(s2 + 1) * 128]
                        nc.tensor.matmul(et, s["kec"][:, qsl], s["qec"][:, qsl],
                                         start=True, stop=False)
                        nc.tensor.matmul(et, s["kes"][:, qsl], s["qes"][:, qsl],
                                         start=False, stop=True)
                    bsl = slice(2 * ms * 128, (2 * ms + 1) * 128)
                    q1sl = slice((2 * ms + 1) * 128, (2 * ms + 2) * 128)
                    et2 = etb[:, 256:384]
                    nc.tensor.matmul(et2, s["kec"][:, bsl], s["qec"][:, q1sl],
                                     start=True, stop=False)
                    nc.tensor.matmul(et2, s["kes"][:, bsl], s["qes"][:, q1sl],
                                     start=False, stop=True)
                    # one masked multiply for all three energy blocks (DVE is
                    # idle during 2b; Pool owns the state bookkeeping)
                    em3 = emp.tile([128, 384], BF16, tag="em3", name=f"em3_{ms}")
                    nc.vector.tensor_mul(em3[:], etb[:, 0:384], mask3[:])
                    return em3

                def emit_ot(ms, em3):
                    qmsl = slice(ms * 256, ms * 256 + 128)
                    smsl = slice(ms * 256 + 128, (ms + 1) * 256)
                    otb = pot.tile([128, 256], F32, tag="otb", name=f"otb{ms}")
                    for s2 in range(2):
                        qb = 2 * ms + s2
                        qsl = slice(qb * 128, (qb + 1) * 128)
                        ot = otb[:, s2 * 128:(s2 + 1) * 128]
                        nc.tensor.matmul(ot, s["QB"][:, qmsl], s["qec"][:, qsl],
                                         start=True, stop=False)
                        nc.tensor.matmul(ot, s["QB"][:, smsl], s["qes"][:, qsl],
                                         start=False, stop=False)
                        if s2 == 1:
                            nc.tensor.matmul(ot, vn_blk(b, 2 * ms),
                                             em3[:, 256:384],
                                             start=False, stop=False)
                        nc.tensor.matmul(ot, vn_blk(b, qb),
                                         em3[:, s2 * 128:(s2 + 1) * 128],
                                         start=False, stop=True)
                    osl = slice(2 * ms * 128, (2 * ms + 2) * 128)
                    nc.scalar.copy(s["yst"][:, osl], otb[:])
                    if ms % 2 == 1:
                        g = ms // 2
                        gsl = slice(g * 512, (g + 1) * 512)
                        nc.sync.dma_start(y_out[b][:, gsl], s["yst"][:, gsl])

                # software pipeline: energies run three chunks ahead of outputs
                DEPTH = 2
                ems = {}
                for ms in range(NS):
                    ems[ms] = emit_et(ms)
                    if ms >= DEPTH:
                        emit_ot(ms - DEPTH, ems.pop(ms - DEPTH))
                for ms in range(NS - DEPTH, NS):
                    emit_ot(ms, ems.pop(ms))

            with tc.tile_pool(name="pq", bufs=2, space="PSUM") as pq, \
                 tc.tile_pool(name="pkv", bufs=3, space="PSUM") as pkv, \
                 tc.tile_pool(name="ptr", bufs=1, space="PSUM") as ptp, \
                 tc.tile_pool(name="pst", bufs=2, space="PSUM") as pst:
                # prefetch b0's x tiles interleaved with table columns
                xts0 = [xt00]
                for t in range(1, 4):
                    xtn = xp.tile([128, 8 * TT], BF16, tag="xt", name=f"xt0{t}")
                    nc.sync.dma_start(xtn[:], xh_in[0][t])
                    emit_tables(t - 1)
                    xts0.append(xtn)
                emit_tables(3)
                # prefetch b1's x tiles too (DMA track is idle during b0)
                xts1 = []
                for t in range(4):
                    xtn = xp.tile([128, 8 * TT], BF16, tag="xt", name=f"xt1{t}")
                    nc.sync.dma_start(xtn[:], xh_in[1][t])
                    xts1.append(xtn)
                wp = pq.tile([128, TT], F32, tag="psq", name="warmps")
                for i in range(7):
                    nc.tensor.matmul(wp[:], wz[:, 0:128], wz[:],
                                     start=True, stop=True)
                # state chunks interleave with projection tiles: chunk 2t
                # and 2t+1 only need tile t's outputs, so the serial P chain
                # advances under the next tile's matmuls
                for t in range(4):
                    phase1_tile(0, t, pq, pkv, ptp, xt=xts0[t])
                    phase2a_chunk(0, 2 * t, pst)
                    phase2a_chunk(0, 2 * t + 1, pst)
                phase2a_post(0)
                for t in range(4):
                    phase1_tile(1, t, pq, pkv, ptp, xt=xts1[t])
                    phase2a_chunk(1, 2 * t, pst)
                    phase2a_chunk(1, 2 * t + 1, pst)
                phase2a_post(1)
            with tc.tile_pool(name="pet", bufs=3, space="PSUM") as pet, \
                 tc.tile_pool(name="pot", bufs=3, space="PSUM") as pot:
                phase2b(0, pet, pot)
                phase2b(1, pet, pot)
            if dbg:
                for nm, dst in dbg_outs.items():
                    nc.sync.dma_start(dst, st[0][nm][:])

    return nc


def build_kernel_b():
    nc = bass.Bass("TRN2", target_bir_lowering=False, debug=False, num_devices=NC)
    NT = B * N // NC  # 512 tokens per core
    yt_in = nc.dram_tensor("ytx", [128, H * NT], BF16, kind="ExternalInput").ap()
    xs_in = nc.dram_tensor("xs", [128, 8 * NT], BF16, kind="ExternalInput").ap()
    wu_in = nc.dram_tensor("wu", [128, 64 * 128], BF16, kind="ExternalInput").ap()
    wo_in = nc.dram_tensor("wo", [128, 64 * 128], BF16, kind="ExternalInput").ap()
    bu_in = nc.dram_tensor("bu", [128, 8], F32, kind="ExternalInput").ap()
    ones_in = nc.dram_tensor("ones", [128, 128], BF16, kind="ExternalInput").ap()
    out_t = nc.dram_tensor("outT", [128, 8 * NT], BF16, kind="ExternalOutput").ap()

    with tile.TileContext(nc) as tc:
        with tc.tile_pool(name="ins", bufs=1) as ip, \
             tc.tile_pool(name="work", bufs=1) as wk, \
             tc.tile_pool(name="y2p", bufs=3) as y2p, \
             tc.tile_pool(name="pu", bufs=4, space="PSUM") as pup, \
             tc.tile_pool(name="po", bufs=3, space="PSUM") as pop, \
             tc.tile_pool(name="pss", bufs=1, space="PSUM") as pss:
            yt = ip.tile([128, H * NT], BF16)
            xs = ip.tile([128, 8 * NT], BF16)
            wut = ip.tile([128, 64 * 128], BF16)
            wot = ip.tile([128, 64 * 128], BF16)
            bu = ip.tile([128, 8], F32)
            ones = ip.tile([128, 128], BF16)
            # DMA order tuned so u-proj starts asap and each consumer's
            # input lands just ahead of its first use
            nc.sync.dma_start(xs[:, 0:4 * NT], xs_in[:, 0:4 * NT])
            nc.sync.dma_start(ones[:], ones_in)
            nc.sync.dma_start(bu[:], bu_in)
            nc.sync.dma_start(wut[:, 0:2048], wu_in[:, 0:2048])
            nc.sync.dma_start(wut[:, 2048:4096], wu_in[:, 2048:4096])
            nc.sync.dma_start(xs[:, 4 * NT:8 * NT], xs_in[:, 4 * NT:8 * NT])
            nc.sync.dma_start(yt[:, 0:4 * NT], yt_in[:, 0:4 * NT])
            nc.sync.dma_start(wut[:, 4096:6144], wu_in[:, 4096:6144])
            nc.sync.dma_start(wut[:, 6144:8192], wu_in[:, 6144:8192])
            nc.sync.dma_start(yt[:, 4 * NT:8 * NT], yt_in[:, 4 * NT:8 * NT])
            for q in range(4):
                nc.sync.dma_start(wot[:, q * 2048:(q + 1) * 2048],
                                  wo_in[:, q * 2048:(q + 1) * 2048])

            # warm the PE p-state ramp during the DMA lead-in
            wz = wk.tile([128, NT], BF16, tag="warm")
            nc.vector.memset(wz[:], 0.0)
            wp = pup.tile([128, NT // 2], F32, tag="pu", name="warmps")
            for i in range(39):
                nc.tensor.matmul(wp[:], wz[:, 0:128], wz[:, 0:NT // 2],
                                 start=True, stop=True)

            # squares queue on Act early (they only need yt halves)
            y2s = []
            for hc in range(H):
                y2 = y2p.tile([128, NT], BF16, tag=f"y2_{hc}", name=f"y2_{hc}")
                nc.scalar.activation(y2[:], yt[:, hc * NT:(hc + 1) * NT], AF.Square)
                y2s.append(y2)

            def wsl(w, a, bb):
                o = (a * 8 + bb) * 128
                return w[:, o:o + 128]

            z = wk.tile([128, 8 * NT], BF16, tag="z")
            ostage = wk.tile([128, 8 * NT], BF16, tag="ostage")
            HT = NT // 2

            def u_half(hf):
                def hslc(base):
                    return slice(base * NT + hf * HT, base * NT + (hf + 1) * HT)
                for uc in range(8):
                    ps = pup.tile([128, HT], F32, tag="pu")
                    for e in range(8):
                        nc.tensor.matmul(ps[:], wsl(wut, uc, e), xs[:, hslc(e)],
                                         start=(e == 0), stop=(e == 7))
                    nc.vector.scalar_tensor_tensor(
                        z[:, hslc(uc)], ps[:], bu[:, uc:uc + 1],
                        yt[:, hslc(uc)], ALU.add, ALU.mult)

            def o_half(hf):
                def hslc(base):
                    return slice(base * NT + hf * HT, base * NT + (hf + 1) * HT)
                for oc in range(8):
                    ps = pop.tile([128, HT], F32, tag="po")
                    for e in range(8):
                        nc.tensor.matmul(ps[:], wsl(wot, oc, e), z[:, hslc(e)],
                                         start=(e == 0), stop=(e == 7))
                    nc.vector.tensor_mul(ostage[:, hslc(oc)], ps[:],
                                         rs2[:, hf * HT:(hf + 1) * HT])
                    if oc % 2 == 1:
                        # ship each finished pair of output blocks
                        o_r = out_t.rearrange("p (a t) -> p a t", a=8)
                        s_r = ostage.rearrange("p (a t) -> p a t", a=8)
                        hsl2 = slice(hf * HT, (hf + 1) * HT)
                        nc.sync.dma_start(o_r[:, oc - 1:oc + 1, hsl2],
                                          s_r[:, oc - 1:oc + 1, hsl2])

            # first token-half u-proj with a split contraction: uc0-3 start
            # on the first xs half + wu quarter while the rest streams in
            def hslc0(base):
                return slice(base * NT, base * NT + HT)
            open_ps = []
            for uc in range(4):
                ps = pup.tile([128, HT], F32, tag="pu", name=f"pu_s{uc}")
                for e in range(4):
                    nc.tensor.matmul(ps[:], wsl(wut, uc, e), xs[:, hslc0(e)],
                                     start=(e == 0), stop=False)
                open_ps.append(ps)
            for uc in range(4):
                ps = open_ps[uc]
                for e in range(4, 8):
                    nc.tensor.matmul(ps[:], wsl(wut, uc, e), xs[:, hslc0(e)],
                                     start=False, stop=(e == 7))
                nc.vector.scalar_tensor_tensor(
                    z[:, hslc0(uc)], ps[:], bu[:, uc:uc + 1],
                    yt[:, hslc0(uc)], ALU.add, ALU.mult)
            for uc in range(4, 8):
                ps = pup.tile([128, HT], F32, tag="pu")
                for e in range(8):
                    nc.tensor.matmul(ps[:], wsl(wut, uc, e), xs[:, hslc0(e)],
                                     start=(e == 0), stop=(e == 7))
                nc.vector.scalar_tensor_tensor(
                    z[:, hslc0(uc)], ps[:], bu[:, uc:uc + 1],
                    yt[:, hslc0(uc)], ALU.add, ALU.mult)
            # ssq + norm chain between the u and o halves: PE fills with
            # these cheap matmuls while wo still streams in
            ssq = pss.tile([128, NT], F32, tag="ssq")
            for hc in range(H):
                nc.tensor.matmul(ssq[:], ones[:], y2s[hc][:],
                                 start=(hc == 0), stop=(hc == H - 1))
            # rsqrt(ms+eps) = exp(-0.5*ln(ms+eps)); one Newton step for accuracy
            xb = wk.tile([128, NT], F32, tag="xb")
            nc.vector.tensor_scalar(xb[:], ssq[:], 1.0 / D, EPS, ALU.mult, ALU.add)
            bz = ip.tile([128, 1], F32)
            nc.vector.memset(bz[:], 0.0)
            lnv = wk.tile([128, NT], F32, tag="lnv")
            nc.scalar.activation(lnv[:], xb[:], AF.Ln, bias=bz[:])
            rsb = wk.tile([128, NT], F32, tag="rsb")
            nc.scalar.activation(rsb[:], lnv[:], AF.Exp, bias=bz[:], scale=-0.5)
            t1 = wk.tile([128, NT], F32, tag="t1")
            nc.vector.tensor_mul(t1[:], xb[:], rsb[:])
            nc.vector.tensor_mul(t1[:], t1[:], rsb[:])
            nc.vector.tensor_scalar(t1[:], t1[:], -0.5, 1.5, ALU.mult, ALU.add)
            rs2 = wk.tile([128, NT], F32, tag="rs2")
            nc.vector.tensor_mul(rs2[:], rsb[:], t1[:])
            u_half(1)
            o_half(0)
            o_half(1)

    return nc


_CACHE = {}


def _bf(a):
    return np.ascontiguousarray(a.astype(BF_NP))


def kernel(x, slope_rate, qkvu_w, qkvu_b, out_w, out_b, theta):
    x = np.asarray(x, np.float32)
    slope_rate = np.asarray(slope_rate, np.float32)
    qkvu_w = np.asarray(qkvu_w, np.float32)
    qkvu_b = np.asarray(qkvu_b, np.float32)
    out_w = np.asarray(out_w, np.float32)
    out_b = np.asarray(out_b, np.float32)
    theta = np.asarray(theta, np.float32)

    # x in [b, t, p, e*512+j] layout: xh[b,t,p,e*512+j] = x[b, t*512+j, e*128+p]
    xh = _bf(x.reshape(B, 4, TT, 8, 128).transpose(0, 1, 4, 3, 2)
             .reshape(B, 4, 128, 8 * TT))
    idx = np.arange(N, dtype=np.float32)
    ident = np.eye(128, dtype=np.float32)

    in_maps_a = []
    for c in range(NC):
        th = theta[c, 0].astype(np.float32)[:, None] * idx[None, :]  # [128, N]
        es = np.exp(slope_rate[c, 0, 0]).astype(np.float32)
        es1 = np.float32(es - 1.0)
        sl = slice(c * HD, (c + 1) * HD)
        wq = qkvu_w[0 * D:1 * D][sl]   # [128, D]
        wk = qkvu_w[1 * D:2 * D][sl]
        wv = qkvu_w[2 * D:3 * D][sl]
        # wq image: [p, e*128+o] = wq[o, e*128+p]
        wq_img = wq.T.reshape(8, 128, 128).transpose(1, 0, 2).reshape(128, 8 * 128)
        # wkv image: [p, e*256 + (k|v 128+o)]
        wkv_img = np.concatenate(
            [wk.T.reshape(8, 128, 1, 128), wv.T.reshape(8, 128, 1, 128)], axis=2
        ).transpose(1, 0, 2, 3).reshape(128, 8 * 256)
        cosv = np.cos(th)  # [hd, pos]
        sinv = np.sin(th)
        # token-layout tables: [p, jj*128 + d] = f(theta_d * (jj*128+p))
        cosvT = cosv.T.reshape(NB, 128, 128).transpose(1, 0, 2).reshape(128, N)
        sinvT = sinv.T.reshape(NB, 128, 128).transpose(1, 0, 2).reshape(128, N)
        m0 = (np.arange(128)[:, None] <= np.arange(128)[None, :]).astype(np.float32)
        m3 = np.concatenate(
            [m0 * es1, m0 * es1, np.full((128, 128), es1, np.float32)], axis=1)
        bkv_img = np.broadcast_to(
            np.concatenate([qkvu_b[1 * D:2 * D][sl], qkvu_b[2 * D:3 * D][sl]] * 2),
            (128, 512))
        in_maps_a.append({
            "xh": xh,
            "wq": _bf(wq_img),
            "wkv": _bf(wkv_img),
            "cosT": _bf(cosv),
            "sinT": _bf(sinv),
            "cosTT": _bf(cosvT),
            "sinTT": _bf(sinvT),
            "mask3": np.ascontiguousarray(m3),
            "bkv": _bf(bkv_img),
            "bq": np.ascontiguousarray(qkvu_b[0 * D:1 * D][sl].reshape(128, 1)),
            "es1": np.full((128, 1), es1, np.float32),
            "ident": _bf(ident),
        })

    if "a" not in _CACHE:
        _CACHE["a"] = build_kernel_a()
    nca = _CACHE["a"]
    if not getattr(nca, "_wsplit_done", False):
        _split_multi_waits(nca)
        nca._wsplit_done = True
    res_a = run_bass_kernel_spmd(nca, in_maps_a, list(range(NC))).results

    # reshard: core d of kernel B gets flat tokens [d*512, (d+1)*512)
    NT = B * N // NC
    xT = x.reshape(B * N, D).T  # [D, B*N]
    wu = qkvu_w[3 * D:4 * D]
    wu_img = wu.reshape(8, 128, 8, 128).transpose(3, 0, 2, 1).reshape(
        128, 64 * 128)  # [p, (uc*8+e)*128+o] = wu[uc*128+o, e*128+p]
    wo_img = out_w.reshape(8, 128, 8, 128).transpose(3, 0, 2, 1).reshape(
        128, 64 * 128)
    bu_img = qkvu_b[3 * D:4 * D].reshape(8, 128).T  # [o, uc]
    onesm = np.ones((128, 128), np.float32)

    in_maps_b = []
    for d in range(NC):
        bb, off = d // 4, (d % 4) * NT
        ytx = np.concatenate(
            [np.asarray(res_a[h]["y"][bb][:, off:off + NT]) for h in range(H)],
            axis=1)  # [128, H*NT] bf16
        xsl = xT[:, d * NT:(d + 1) * NT].reshape(8, 128, NT).transpose(
            1, 0, 2).reshape(128, 8 * NT)
        in_maps_b.append({
            "ytx": np.ascontiguousarray(ytx),
            "xs": _bf(xsl),
            "wu": _bf(wu_img),
            "wo": _bf(wo_img),
            "bu": np.ascontiguousarray(bu_img),
            "ones": _bf(onesm),
        })

    if "b" not in _CACHE:
        _CACHE["b"] = build_kernel_b()
    ncb = _CACHE["b"]
    if not getattr(ncb, "_wsplit_done", False):
        _split_multi_waits(ncb)
        ncb._wsplit_done = True
    res_b = run_bass_kernel_spmd(ncb, in_maps_b, list(range(NC))).results

    out = np.empty((B * N, D), np.float32)
    for d in range(NC):
        o = np.asarray(res_b[d]["outT"]).astype(np.float32)  # [128, 8*NT]
        out[d * NT:(d + 1) * NT] = o.reshape(128, 8, NT).transpose(
            2, 1, 0).reshape(NT, D)
    out += out_b[None, :]
    return out.reshape(B, N, D)


# revision 80
# speedup vs baseline: 1.0393x; 1.0106x over previous
"""NormLinearAttention kernel for 8 Trainium2 NeuronCores (bf16 build).

Strategy:
  Kernel A: head-parallel (core c owns head c, both batches). bf16 matmuls
    (1 cyc/row at any free size on TRN2). Projections: q in [hd, tok]
    layout; k|v paired in one token-layout matmul so v needs no transpose.
    Linear attention with mask = exp(s*tril):
        out_i = (e^s-1)*causal_i + full_i
    chunked at 128 (q blocks) with state snapshots every 256 tokens:
        Q_m = (e^s-1)*P_m + S_full  (precombined, bf16)
        ot  = v_blk @ (mask.et) [+ v_prev @ es1.et_off] + Q_m^T q
  Host reshards head-parallel y -> token-parallel for Kernel B.
  Kernel B: token-parallel (512 tokens/core): u proj, SRMSNorm via Rsqrt
    activation, z = (u+bu)*y, out proj, scale by rsqrt.

Both batches' phases are interleaved so DVE/Pool bookkeeping chains hide
under the other batch's PE work. DMAs are few and large (one per x tile).
"""

import sys

sys.path.insert(0, "/opt/trn_rl_repo")

import numpy as np
import ml_dtypes

import bass_rust
import concourse.bass as bass
import concourse.mybir as mybir
import concourse.tile as tile
from concourse.bass_utils import run_bass_kernel_spmd

F32 = mybir.dt.float32
BF16 = mybir.dt.bfloat16
AF = mybir.ActivationFunctionType
ALU = mybir.AluOpType

B, N, D = 2, 2048, 1024
H = 8
HD = 128
NC = 8
TT = 512           # projection token tile
NB = N // 128      # 16 token blocks per batch
NS = N // 256      # 8 state chunks per batch
EPS = 1e-6
BF_NP = ml_dtypes.bfloat16


def _split_multi_waits(nc, max_waits=1):
    """walrus accepts one sync wait per instruction; Tile emits several.
    Insert same-engine NoOps each carrying one of the extra waits."""
    ctr = 0
    for func in nc.m.functions:
        for blk in func.blocks:
            out = []
            changed = False
            for inst in blk.instructions:
                si = inst.sync_info
                if si is not None and len(si.on_wait) > max_waits:
                    waits = list(si.on_wait)
                    for w in waits[:-max_waits]:
                        nop = bass_rust.InstNoOp(name=f"WSPLIT-{ctr}", ins=[], outs=[])
                        ctr += 1
                        nop.engine = inst.engine
                        nop.sync_info = mybir.SyncInfo(on_wait=[w], on_update=[])
                        out.append(nop)
                    inst.sync_info = mybir.SyncInfo(
                        on_wait=waits[-max_waits:], on_update=list(si.on_update)
                    )
                    changed = True
                out.append(inst)
            if changed:
                blk.instructions = out
    return ctr


def build_kernel_a(dbg=False):
    nc = bass.Bass("TRN2", target_bir_lowering=False, debug=False, num_devices=NC)
    xh_in = nc.dram_tensor("xh", [B, 4, 128, 8 * TT], BF16, kind="ExternalInput").ap()
    wq_in = nc.dram_tensor("wq", [128, 8 * 128], BF16, kind="ExternalInput").ap()
    wkv_in = nc.dram_tensor("wkv", [128, 8 * 256], BF16, kind="ExternalInput").ap()
    cosT_in = nc.dram_tensor("cosT", [128, N], BF16, kind="ExternalInput").ap()
    sinT_in = nc.dram_tensor("sinT", [128, N], BF16, kind="ExternalInput").ap()
    cosTT_in = nc.dram_tensor("cosTT", [128, N], BF16, kind="ExternalInput").ap()
    sinTT_in = nc.dram_tensor("sinTT", [128, N], BF16, kind="ExternalInput").ap()
    mask3_in = nc.dram_tensor("mask3", [128, 384], F32, kind="ExternalInput").ap()
    bkv_in = nc.dram_tensor("bkv", [128, 512], BF16, kind="ExternalInput").ap()
    bq_in = nc.dram_tensor("bq", [128, 1], F32, kind="ExternalInput").ap()
    es1_in = nc.dram_tensor("es1", [128, 1], F32, kind="ExternalInput").ap()
    ident_in = nc.dram_tensor("ident", [128, 128], BF16, kind="ExternalInput").ap()
    y_out = nc.dram_tensor("y", [B, 128, N], BF16, kind="ExternalOutput").ap()
    dbg_outs = {}
    if dbg:
        for nm in ("qec", "qes", "kec", "kes", "knc", "kns", "ksn", "ksT"):
            dbg_outs[nm] = nc.dram_tensor(
                "d_" + nm, [128, N], BF16, kind="ExternalOutput").ap()
        dbg_outs["kvnat"] = nc.dram_tensor(
            "d_kvnat", [128, NB * 256], BF16, kind="ExternalOutput").ap()
        dbg_outs["QB"] = nc.dram_tensor(
            "d_QB", [128, NS * 256], BF16, kind="ExternalOutput").ap()

    with tile.TileContext(nc) as tc:
        with tc.tile_pool(name="consts", bufs=1) as cp, \
             tc.tile_pool(name="big", bufs=1) as bp, \
             tc.tile_pool(name="xt", bufs=8) as xp, \
             tc.tile_pool(name="qsil", bufs=3) as qsp, \
             tc.tile_pool(name="em", bufs=6) as emp:
            # wq first (smallest), then the first x tile in quarters so the
            # first projection matmuls start as early as possible
            xt00 = xp.tile([128, 8 * TT], BF16, tag="xt")
            wqt = cp.tile([128, 8 * 128], BF16)
            wkvt = cp.tile([128, 8 * 256], BF16)
            cosT = cp.tile([128, N], BF16)
            sinT = cp.tile([128, N], BF16)
            cosTT = cp.tile([128, N], BF16)
            sinTT = cp.tile([128, N], BF16)
            mask3 = cp.tile([128, 384], F32)
            bkv = cp.tile([128, 512], BF16)
            bq = cp.tile([128, 1], F32)
            es1 = cp.tile([128, 1], F32)
            ident = cp.tile([128, 128], BF16)
            bz = cp.tile([128, 1], F32)
            nc.sync.dma_start(wqt[:], wq_in)
            nc.sync.dma_start(xt00[:, 0:2 * TT], xh_in[0][0][:, 0:2 * TT])
            nc.sync.dma_start(xt00[:, 2 * TT:4 * TT], xh_in[0][0][:, 2 * TT:4 * TT])
            nc.sync.dma_start(wkvt[:], wkv_in)
            nc.sync.dma_start(xt00[:, 4 * TT:8 * TT], xh_in[0][0][:, 4 * TT:8 * TT])
            nc.sync.dma_start(bq[:], bq_in)
            nc.sync.dma_start(bkv[:], bkv_in)
            nc.sync.dma_start(es1[:], es1_in)
            nc.sync.dma_start(ident[:], ident_in)
            nc.vector.memset(bz[:], 0.0)
            # warm the PE p-state ramp while the first DMAs land: throwaway
            # matmuls on a zeroed tile so real matmuls start at full speed
            wz = qsp.tile([128, TT], BF16, tag="warm")
            nc.vector.memset(wz[:], 0.0)

            st = []  # per-batch persistent tensors
            for b in range(B):
                d = {}
                for nm, shape, dt in (
                    ("kvnat", [128, NB * 256], BF16),
                    ("ksn", [128, N], BF16),
                    ("ksT", [128, N], BF16),
                    ("qec", [128, N], BF16),
                    ("qes", [128, N], BF16),
                    ("kec", [128, N], BF16),
                    ("kes", [128, N], BF16),
                    ("knc", [128, N], BF16),
                    ("kns", [128, N], BF16),
                    ("QB", [128, NS * 256], BF16),
                    ("Pb", [128, 256], F32),
                    ("yst", [128, N], BF16),
                ):
                    d[nm] = bp.tile(shape, dt, tag=f"{nm}{b}", name=f"{nm}{b}")
                st.append(d)

            def vn_blk(b, j):
                # v for token block j lives in the kv-paired tile
                return st[b]["kvnat"][:, j * 256 + 128:(j + 1) * 256]

            def emit_tables(t):
                # stream each table's columns for one tile, so x tiles and
                # tables interleave on the DMA queue
                tsl = slice(t * TT, (t + 1) * TT)
                nc.sync.dma_start(cosT[:, tsl], cosT_in[:, tsl])
                nc.sync.dma_start(sinT[:, tsl], sinT_in[:, tsl])
                nc.sync.dma_start(cosTT[:, tsl], cosTT_in[:, tsl])
                nc.sync.dma_start(sinTT[:, tsl], sinTT_in[:, tsl])

            def phase1_tile(b, t, pq, pkv, ptp, xt=None):
                s = st[b]
                tsl = slice(t * TT, (t + 1) * TT)
                if xt is None:
                    xt = xp.tile([128, 8 * TT], BF16, tag="xt")
                    nc.sync.dma_start(xt[:], xh_in[b][t])
                # q projection: [hd, tok]
                psq = pq.tile([128, TT], F32, tag="psq")
                for e in range(8):
                    nc.tensor.matmul(
                        psq[:], wqt[:, e * 128:(e + 1) * 128],
                        xt[:, e * TT:(e + 1) * TT],
                        start=(e == 0), stop=(e == 7))
                qsil = qsp.tile([128, TT], BF16, tag="qsil" if b == 0 else f"qsil1_{t}",
                                bufs=1 if b else None)
                nc.scalar.activation(qsil[:], psq[:], AF.Silu, bias=bq[:])
                # k|v paired projection: [tok, k|v]; two token blocks share a
                # psum bank as sequential accumulation groups
                ptr4 = ptp.tile([128, 512], BF16, tag="ptr4")
                for jp in range(2):
                    pskv = pkv.tile([128, 512], F32, tag="pskv")
                    for half in range(2):
                        jb = jp * 2 + half
                        for e in range(8):
                            nc.tensor.matmul(
                                pskv[:, half * 256:(half + 1) * 256],
                                xt[:, e * TT + jb * 128: e * TT + (jb + 1) * 128],
                                wkvt[:, e * 256:(e + 1) * 256],
                                start=(e == 0), stop=(e == 7))
                    jj0 = t * 4 + jp * 2
                    nc.vector.tensor_add(
                        s["kvnat"][:, jj0 * 256:(jj0 + 2) * 256], pskv[:], bkv[:])
                    kpair = s["kvnat"].rearrange("p (j kv c) -> p j kv c",
                                                 kv=2, c=128)
                    nc.scalar.activation(
                        s["ksn"][:, jj0 * 128:(jj0 + 2) * 128],
                        kpair[:, jj0:jj0 + 2, 0],
                        AF.Silu, bias=bz[:])
                    for half in range(2):
                        jb = jp * 2 + half
                        jj = t * 4 + jb
                        nc.tensor.transpose(ptr4[:, jb * 128:(jb + 1) * 128],
                                            s["ksn"][:, jj * 128:(jj + 1) * 128],
                                            ident[:])
                nc.scalar.copy(s["ksT"][:, tsl], ptr4[:])
                # lrpe, state-path (knc/kns) first: the interleaved 2a chunk
                # matmuls wait on them; q/kec/kes are only read in 2b.  For
                # b=1 the 2b-only muls are deferred so they don't sit in the
                # DVE queue ahead of 2b(0)'s masked multiplies.
                nc.vector.tensor_mul(s["knc"][:, tsl], s["ksn"][:, tsl], cosTT[:, tsl])
                nc.vector.tensor_mul(s["kns"][:, tsl], s["ksn"][:, tsl], sinTT[:, tsl])

                def lrpe_2b():
                    nc.vector.tensor_mul(s["kec"][:, tsl], s["ksT"][:, tsl], cosT[:, tsl])
                    nc.vector.tensor_mul(s["kes"][:, tsl], s["ksT"][:, tsl], sinT[:, tsl])
                    nc.vector.tensor_mul(s["qec"][:, tsl], qsil[:], cosT[:, tsl])
                    nc.vector.tensor_mul(s["qes"][:, tsl], qsil[:], sinT[:, tsl])
                if b == 0:
                    lrpe_2b()
                    return None
                return lrpe_2b

            def phase2a_chunk(b, ms, pst):
                s = st[b]
                if ms == 0:
                    nc.gpsimd.memset(s["Pb"][:], 0.0)
                else:
                    # snapshot (e^s-1)*P_m before adding chunk ms
                    nc.gpsimd.tensor_scalar_mul(
                        s["QB"][:, ms * 256:(ms + 1) * 256], s["Pb"][:], es1[:])
                scb = pst.tile([128, 256], F32, tag="scb")
                # two sequential accumulation groups sharing one bank
                for g in range(2):
                    kn = s["knc"] if g == 0 else s["kns"]
                    for s2 in range(2):
                        j = 2 * ms + s2
                        bsl = slice(j * 128, (j + 1) * 128)
                        nc.tensor.matmul(scb[:, g * 128:(g + 1) * 128],
                                         kn[:, bsl], vn_blk(b, j),
                                         start=(s2 == 0), stop=(s2 == 1))
                # gpsimd cannot touch PSUM on hw; this add reads scb (PSUM)
                nc.vector.tensor_add(s["Pb"][:], s["Pb"][:], scb[:])

            def phase2a_post(b):
                s = st[b]
                # Q_0 = S_full; Q_m += S_full
                nc.gpsimd.tensor_copy(s["QB"][:, 0:256], s["Pb"][:])
                for ms in range(1, NS):
                    msl = slice(ms * 256, (ms + 1) * 256)
                    nc.gpsimd.tensor_add(s["QB"][:, msl], s["QB"][:, msl],
                                         s["QB"][:, 0:256])

            def phase2b(b, pet, pot):
                s = st[b]

                def emit_et(ms):
                    etb = pet.tile([128, 512], F32, tag="etb", name=f"etb{ms}")
                    for s2 in range(2):
                        qb = 2 * ms + s2
                        qsl = slice(qb * 128, (qb + 1) * 128)
                        et = etb[:, s2 * 128:(s2 + 1) * 128]
                        nc.tensor.matmul(et, s["kec"][:, qsl], s["qec"][:, qsl],
                                         start=True, stop=False)
                        nc.tensor.matmul(et, s["kes"][:, qsl], s["qes"][:, qsl],
                                         start=False, stop=True)
                    bsl = slice(2 * ms * 128, (2 * ms + 1) * 128)
                    q1sl = slice((2 * ms + 1) * 128, (2 * ms + 2) * 128)
                    et2 = etb[:, 256:384]
                    nc.tensor.matmul(et2, s["kec"][:, bsl], s["qec"][:, q1sl],
                                     start=True, stop=False)
                    nc.tensor.matmul(et2, s["kes"][:, bsl], s["qes"][:, q1sl],
                                     start=False, stop=True)
                    # one masked multiply for all three energy blocks (DVE is
                    # idle during 2b; Pool owns the state bookkeeping)
                    em3 = emp.tile([128, 384], BF16, tag="em3", name=f"em3_{ms}")
                    nc.vector.tensor_mul(em3[:], etb[:, 0:384], mask3[:])
                    return em3

                def emit_ot(ms, em3):
                    qmsl = slice(ms * 256, ms * 256 + 128)
                    smsl = slice(ms * 256 + 128, (ms + 1) * 256)
                    otb = pot.tile([128, 256], F32, tag="otb", name=f"otb{ms}")
                    for s2 in range(2):
                        qb = 2 * ms + s2
                        qsl = slice(qb * 128, (qb + 1) * 128)
                        ot = otb[:, s2 * 128:(s2 + 1) * 128]
                        nc.tensor.matmul(ot, s["QB"][:, qmsl], s["qec"][:, qsl],
                                         start=True, stop=False)
                        nc.tensor.matmul(ot, s["QB"][:, smsl], s["qes"][:, qsl],
                                         start=False, stop=False)
                        if s2 == 1:
                            nc.tensor.matmul(ot, vn_blk(b, 2 * ms),
                                             em3[:, 256:384],
                                             start=False, stop=False)
                        nc.tensor.matmul(ot, vn_blk(b, qb),
                                         em3[:, s2 * 128:(s2 + 1) * 128],
                                         start=False, stop=True)
                    osl = slice(2 * ms * 128, (2 * ms + 2) * 128)
                    nc.scalar.copy(s["yst"][:, osl], otb[:])
                    if ms % 2 == 1:
                        g = ms // 2
                        gsl = slice(g * 512, (g + 1) * 512)
                        nc.sync.dma_start(y_out[b][:, gsl], s["yst"][:, gsl])

                # software pipeline: energies run three chunks ahead of outputs
                DEPTH = 2
                ems = {}
                for ms in range(NS):
                    ems[ms] = emit_et(ms)
                    if ms >= DEPTH:
                        emit_ot(ms - DEPTH, ems.pop(ms - DEPTH))
                for ms in range(NS - DEPTH, NS):
                    emit_ot(ms, ems.pop(ms))

            with tc.tile_pool(name="pq", bufs=2, space="PSUM") as pq, \
                 tc.tile_pool(name="pkv", bufs=3, space="PSUM") as pkv, \
                 tc.tile_pool(name="ptr", bufs=1, space="PSUM") as ptp, \
                 tc.tile_pool(name="pst", bufs=2, space="PSUM") as pst:
                # prefetch b0's x tiles interleaved with table columns
                xts0 = [xt00]
                for t in range(1, 4):
                    xtn = xp.tile([128, 8 * TT], BF16, tag="xt", name=f"xt0{t}")
                    nc.sync.dma_start(xtn[:], xh_in[0][t])
                    emit_tables(t - 1)
                    xts0.append(xtn)
                emit_tables(3)
                # prefetch b1's x tiles too (DMA track is idle during b0)
                xts1 = []
                for t in range(4):
                    xtn = xp.tile([128, 8 * TT], BF16, tag="xt", name=f"xt1{t}")
                    nc.sync.dma_start(xtn[:], xh_in[1][t])
                    xts1.append(xtn)
                nc.sync.dma_start(mask3[:], mask3_in)
                wp = pq.tile([128, TT], F32, tag="psq", name="warmps")
                for i in range(7):
                    nc.tensor.matmul(wp[:], wz[:, 0:128], wz[:],
                                     start=True, stop=True)
                # state chunks interleave with projection tiles: chunk 2t
                # and 2t+1 only need tile t's outputs, so the serial P chain
                # advances under the next tile's matmuls
                for t in range(4):
                    phase1_tile(0, t, pq, pkv, ptp, xt=xts0[t])
                    phase2a_chunk(0, 2 * t, pst)
                    phase2a_chunk(0, 2 * t + 1, pst)
                phase2a_post(0)
                deferred = []
                for t in range(4):
                    deferred.append(
                        phase1_tile(1, t, pq, pkv, ptp, xt=xts1[t]))
                    phase2a_chunk(1, 2 * t, pst)
                    phase2a_chunk(1, 2 * t + 1, pst)
                phase2a_post(1)
            with tc.tile_pool(name="pet", bufs=3, space="PSUM") as pet, \
                 tc.tile_pool(name="pot", bufs=3, space="PSUM") as pot:
                phase2b(0, pet, pot)
                # b1's 2b-only lrpe muls run on DVE while 2b(0) computes
                for fn in deferred:
                    fn()
                phase2b(1, pet, pot)
            if dbg:
                for nm, dst in dbg_outs.items():
                    nc.sync.dma_start(dst, st[0][nm][:])

    return nc


def build_kernel_b():
    nc = bass.Bass("TRN2", target_bir_lowering=False, debug=False, num_devices=NC)
    NT = B * N // NC  # 512 tokens per core
    yt_in = nc.dram_tensor("ytx", [128, H * NT], BF16, kind="ExternalInput").ap()
    xs_in = nc.dram_tensor("xs", [128, 8 * NT], BF16, kind="ExternalInput").ap()
    wu_in = nc.dram_tensor("wu", [128, 64 * 128], BF16, kind="ExternalInput").ap()
    wo_in = nc.dram_tensor("wo", [128, 64 * 128], BF16, kind="ExternalInput").ap()
    bu_in = nc.dram_tensor("bu", [128, 8], F32, kind="ExternalInput").ap()
    ones_in = nc.dram_tensor("ones", [128, 128], BF16, kind="ExternalInput").ap()
    out_t = nc.dram_tensor("outT", [128, 8 * NT], BF16, kind="ExternalOutput").ap()

    with tile.TileContext(nc) as tc:
        with tc.tile_pool(name="ins", bufs=1) as ip, \
             tc.tile_pool(name="work", bufs=1) as wk, \
             tc.tile_pool(name="y2p", bufs=3) as y2p, \
             tc.tile_pool(name="pu", bufs=4, space="PSUM") as pup, \
             tc.tile_pool(name="po", bufs=3, space="PSUM") as pop, \
             tc.tile_pool(name="pss", bufs=1, space="PSUM") as pss:
            yt = ip.tile([128, H * NT], BF16)
            xs = ip.tile([128, 8 * NT], BF16)
            wut = ip.tile([128, 64 * 128], BF16)
            wot = ip.tile([128, 64 * 128], BF16)
            bu = ip.tile([128, 8], F32)
            ones = ip.tile([128, 128], BF16)
            # DMA order tuned so u-proj starts asap and each consumer's
            # input lands just ahead of its first use
            nc.sync.dma_start(xs[:, 0:4 * NT], xs_in[:, 0:4 * NT])
            nc.sync.dma_start(ones[:], ones_in)
            nc.sync.dma_start(bu[:], bu_in)
            nc.sync.dma_start(wut[:, 0:2048], wu_in[:, 0:2048])
            nc.sync.dma_start(wut[:, 2048:4096], wu_in[:, 2048:4096])
            nc.sync.dma_start(xs[:, 4 * NT:8 * NT], xs_in[:, 4 * NT:8 * NT])
            nc.sync.dma_start(yt[:, 0:4 * NT], yt_in[:, 0:4 * NT])
            nc.sync.dma_start(wut[:, 4096:6144], wu_in[:, 4096:6144])
            nc.sync.dma_start(wut[:, 6144:8192], wu_in[:, 6144:8192])
            nc.sync.dma_start(yt[:, 4 * NT:8 * NT], yt_in[:, 4 * NT:8 * NT])
            for q in range(4):
                nc.sync.dma_start(wot[:, q * 2048:(q + 1) * 2048],
                                  wo_in[:, q * 2048:(q + 1) * 2048])

            # warm the PE p-state ramp during the DMA lead-in
            wz = wk.tile([128, NT], BF16, tag="warm")
            nc.vector.memset(wz[:], 0.0)
            wp = pup.tile([128, NT // 2], F32, tag="pu", name="warmps")
            for i in range(39):
                nc.tensor.matmul(wp[:], wz[:, 0:128], wz[:, 0:NT // 2],
                                 start=True, stop=True)

            # squares queue on Act early (they only need yt halves)
            y2s = []
            for hc in range(H):
                y2 = y2p.tile([128, NT], BF16, tag=f"y2_{hc}", name=f"y2_{hc}")
                nc.scalar.activation(y2[:], yt[:, hc * NT:(hc + 1) * NT], AF.Square)
                y2s.append(y2)

            def wsl(w, a, bb):
                o = (a * 8 + bb) * 128
                return w[:, o:o + 128]

            z = wk.tile([128, 8 * NT], BF16, tag="z")
            ostage = wk.tile([128, 8 * NT], BF16, tag="ostage")
            HT = NT // 2

            def u_half(hf):
                def hslc(base):
                    return slice(base * NT + hf * HT, base * NT + (hf + 1) * HT)
                for uc in range(8):
                    ps = pup.tile([128, HT], F32, tag="pu")
                    for e in range(8):
                        nc.tensor.matmul(ps[:], wsl(wut, uc, e), xs[:, hslc(e)],
                                         start=(e == 0), stop=(e == 7))
                    nc.vector.scalar_tensor_tensor(
                        z[:, hslc(uc)], ps[:], bu[:, uc:uc + 1],
                        yt[:, hslc(uc)], ALU.add, ALU.mult)

            def o_half(hf):
                def hslc(base):
                    return slice(base * NT + hf * HT, base * NT + (hf + 1) * HT)
                for oc in range(8):
                    ps = pop.tile([128, HT], F32, tag="po")
                    for e in range(8):
                        nc.tensor.matmul(ps[:], wsl(wot, oc, e), z[:, hslc(e)],
                                         start=(e == 0), stop=(e == 7))
                    nc.vector.tensor_mul(ostage[:, hslc(oc)], ps[:],
                                         rs2[:, hf * HT:(hf + 1) * HT])
                    if oc % 2 == 1:
                        # ship each finished pair of output blocks
                        o_r = out_t.rearrange("p (a t) -> p a t", a=8)
                        s_r = ostage.rearrange("p (a t) -> p a t", a=8)
                        hsl2 = slice(hf * HT, (hf + 1) * HT)
                        nc.sync.dma_start(o_r[:, oc - 1:oc + 1, hsl2],
                                          s_r[:, oc - 1:oc + 1, hsl2])

            # first token-half u-proj with a split contraction: uc0-3 start
            # on the first xs half + wu quarter while the rest streams in
            def hslc0(base):
                return slice(base * NT, base * NT + HT)
            open_ps = []
            for uc in range(4):
                ps = pup.tile([128, HT], F32, tag="pu", name=f"pu_s{uc}")
                for e in range(4):
                    nc.tensor.matmul(ps[:], wsl(wut, uc, e), xs[:, hslc0(e)],
                                     start=(e == 0), stop=False)
                open_ps.append(ps)
            for uc in range(4):
                ps = open_ps[uc]
                for e in range(4, 8):
                    nc.tensor.matmul(ps[:], wsl(wut, uc, e), xs[:, hslc0(e)],
                                     start=False, stop=(e == 7))
                nc.vector.scalar_tensor_tensor(
                    z[:, hslc0(uc)], ps[:], bu[:, uc:uc + 1],
                    yt[:, hslc0(uc)], ALU.add, ALU.mult)
            for uc in range(4, 8):
                ps = pup.tile([128, HT], F32, tag="pu")
                for e in range(8):
                    nc.tensor.matmul(ps[:], wsl(wut, uc, e), xs[:, hslc0(e)],
                                     start=(e == 0), stop=(e == 7))
                nc.vector.scalar_tensor_tensor(
                    z[:, hslc0(uc)], ps[:], bu[:, uc:uc + 1],
                    yt[:, hslc0(uc)], ALU.add, ALU.mult)
            # ssq + norm chain between the u and o halves: PE fills with
            # these cheap matmuls while wo still streams in
            ssq = pss.tile([128, NT], F32, tag="ssq")
            for hc in range(H):
                nc.tensor.matmul(ssq[:], ones[:], y2s[hc][:],
                                 start=(hc == 0), stop=(hc == H - 1))
            # rsqrt(ms+eps) = exp(-0.5*ln(ms+eps)); one Newton step for accuracy
            xb = wk.tile([128, NT], F32, tag="xb")
            nc.vector.tensor_scalar(xb[:], ssq[:], 1.0 / D, EPS, ALU.mult, ALU.add)
            bz = ip.tile([128, 1], F32)
            nc.vector.memset(bz[:], 0.0)
            lnv = wk.tile([128, NT], F32, tag="lnv")
            nc.scalar.activation(lnv[:], xb[:], AF.Ln, bias=bz[:])
            rsb = wk.tile([128, NT], F32, tag="rsb")
            nc.scalar.activation(rsb[:], lnv[:], AF.Exp, bias=bz[:], scale=-0.5)
            t1 = wk.tile([128, NT], F32, tag="t1")
            nc.vector.tensor_mul(t1[:], xb[:], rsb[:])
            nc.vector.tensor_mul(t1[:], t1[:], rsb[:])
            nc.vector.tensor_scalar(t1[:], t1[:], -0.5, 1.5, ALU.mult, ALU.add)
            rs2 = wk.tile([128, NT], F32, tag="rs2")
            nc.vector.tensor_mul(rs2[:], rsb[:], t1[:])
            u_half(1)
            o_half(0)
            o_half(1)

    return nc


_CACHE = {}


def _bf(a):
    return np.ascontiguousarray(a.astype(BF_NP))


def kernel(x, slope_rate, qkvu_w, qkvu_b, out_w, out_b, theta):
    x = np.asarray(x, np.float32)
    slope_rate = np.asarray(slope_rate, np.float32)
    qkvu_w = np.asarray(qkvu_w, np.float32)
    qkvu_b = np.asarray(qkvu_b, np.float32)
    out_w = np.asarray(out_w, np.float32)
    out_b = np.asarray(out_b, np.float32)
    theta = np.asarray(theta, np.float32)

    # x in [b, t, p, e*512+j] layout: xh[b,t,p,e*512+j] = x[b, t*512+j, e*128+p]
    xh = _bf(x.reshape(B, 4, TT, 8, 128).transpose(0, 1, 4, 3, 2)
             .reshape(B, 4, 128, 8 * TT))
    idx = np.arange(N, dtype=np.float32)
    ident = np.eye(128, dtype=np.float32)

    in_maps_a = []
    for c in range(NC):
        th = theta[c, 0].astype(np.float32)[:, None] * idx[None, :]  # [128, N]
        es = np.exp(slope_rate[c, 0, 0]).astype(np.float32)
        es1 = np.float32(es - 1.0)
        sl = slice(c * HD, (c + 1) * HD)
        wq = qkvu_w[0 * D:1 * D][sl]   # [128, D]
        wk = qkvu_w[1 * D:2 * D][sl]
        wv = qkvu_w[2 * D:3 * D][sl]
        # wq image: [p, e*128+o] = wq[o, e*128+p]
        wq_img = wq.T.reshape(8, 128, 128).transpose(1, 0, 2).reshape(128, 8 * 128)
        # wkv image: [p, e*256 + (k|v 128+o)]
        wkv_img = np.concatenate(
            [wk.T.reshape(8, 128, 1, 128), wv.T.reshape(8, 128, 1, 128)], axis=2
        ).transpose(1, 0, 2, 3).reshape(128, 8 * 256)
        cosv = np.cos(th)  # [hd, pos]
        sinv = np.sin(th)
        # token-layout tables: [p, jj*128 + d] = f(theta_d * (jj*128+p))
        cosvT = cosv.T.reshape(NB, 128, 128).transpose(1, 0, 2).reshape(128, N)
        sinvT = sinv.T.reshape(NB, 128, 128).transpose(1, 0, 2).reshape(128, N)
        m0 = (np.arange(128)[:, None] <= np.arange(128)[None, :]).astype(np.float32)
        m3 = np.concatenate(
            [m0 * es1, m0 * es1, np.full((128, 128), es1, np.float32)], axis=1)
        bkv_img = np.broadcast_to(
            np.concatenate([qkvu_b[1 * D:2 * D][sl], qkvu_b[2 * D:3 * D][sl]] * 2),
            (128, 512))
        in_maps_a.append({
            "xh": xh,
            "wq": _bf(wq_img),
            "wkv": _bf(wkv_img),
            "cosT": _bf(cosv),
            "sinT": _bf(sinv),
            "cosTT": _bf(cosvT),
            "sinTT": _bf(sinvT),
            "mask3": np.ascontiguousarray(m3),
            "bkv": _bf(bkv_img),
            "bq": np.ascontiguousarray(qkvu_b[0 * D:1 * D][sl].reshape(128, 1)),
            "es1": np.full((128, 1), es1, np.float32),
            "ident": _bf(ident),
        })

    if "a" not in _CACHE:
        _CACHE["a"] = build_kernel_a()
    nca = _CACHE["a"]
    if not getattr(nca, "_wsplit_done", False):
        _split_multi_waits(nca)
        nca._wsplit_done = True
    res_a = run_bass_kernel_spmd(nca, in_maps_a, list(range(NC))).results

    # reshard: core d of kernel B gets flat tokens [d*512, (d+1)*512)
    NT = B * N // NC
    xT = x.reshape(B * N, D).T  # [D, B*N]
    wu = qkvu_w[3 * D:4 * D]
    wu_img = wu.reshape(8, 128, 8, 128).transpose(3, 0, 2, 1).reshape(
        128, 64 * 128)  # [p, (uc*8+e)*128+o] = wu[uc*128+o, e*128+p]
    wo_img = out_w.reshape(8, 128, 8, 128).transpose(3, 0, 2, 1).reshape(
        128, 64 * 128)
    bu_img = qkvu_b[3 * D:4 * D].reshape(8, 128).T  # [o, uc]
    onesm = np.ones((128, 128), np.float32)

    in_maps_b = []
    for d in range(NC):
        bb, off = d // 4, (d % 4) * NT
        ytx = np.concatenate(
            [np.asarray(res_a[h]["y"][bb][:, off:off + NT]) for h in range(H)],
            axis=1)  # [128, H*NT] bf16
        xsl = xT[:, d * NT:(d + 1) * NT].reshape(8, 128, NT).transpose(
            1, 0, 2).reshape(128, 8 * NT)
        in_maps_b.append({
            "ytx": np.ascontiguousarray(ytx),
            "xs": _bf(xsl),
            "wu": _bf(wu_img),
            "wo": _bf(wo_img),
            "bu": np.ascontiguousarray(bu_img),
            "ones": _bf(onesm),
        })

    if "b" not in _CACHE:
        _CACHE["b"] = build_kernel_b()
    ncb = _CACHE["b"]
    if not getattr(ncb, "_wsplit_done", False):
        _split_multi_waits(ncb)
        ncb._wsplit_done = True
    res_b = run_bass_kernel_spmd(ncb, in_maps_b, list(range(NC))).results

    out = np.empty((B * N, D), np.float32)
    for d in range(NC):
        o = np.asarray(res_b[d]["outT"]).astype(np.float32)  # [128, 8*NT]
        out[d * NT:(d + 1) * NT] = o.reshape(128, 8, NT).transpose(
            2, 1, 0).reshape(NT, D)
    out += out_b[None, :]
    return out.reshape(B, N, D)
